# revision 5
# baseline (speedup 1.0000x reference)
"""Multi-head attention block (QKV proj + SDPA + merge-scramble + fc +
residual + LayerNorm) on 8 Trainium2 NeuronCores.

Sharding: data-parallel over the flattened batch dim (b*n = 32 sequences),
4 sequences per core. Each core runs an identical Bass program on its shard.

Per-sequence math (t = d = e = 512, H = 8 heads, dk = dv = 64):
  Q = qf @ w_q.T ; K = kf @ w_k.T ; V = vf @ w_v.T
  S_h = (Q_h K_h^T) / 8 ;  A_h = softmax(S_h) ;  O_h = A_h V_h
  x = merge_heads(O)            # [t, e]
  x = x.T (the reference's transpose+view scramble; legal since t == e)
  y = LN(x @ w_fc.T + qf) * gamma + beta

On-chip layout strategy: compute Q^T/K^T ([e, t], head-major on partitions)
and V ([t, e] with a per-head ones column) so that S^T = K_h Q_h^T comes out
with tk on partitions. Softmax needs no max-subtraction (|S/8| < ~7) and no
partition reduction: exp runs elementwise on ScalarE over 2-PSUM-bank
[128,1024] tiles. The AV matmuls then run in NATURAL orientation
(lhsT = expS^T chunk [tk,tq], rhs = V_h||ones [tk,65]) producing O[tq,dv]
per (head, tq-chunk) packed 4 heads to a PSUM bank; the ones column yields
the softmax denominator per tq row in the same tile, so normalization is a
single strided multiply into x (natural layout, tq on partitions) — no PE
transposes anywhere. The scramble means fc contracts over the *time* index,
which is exactly x natural as lhsT.

Matmuls run in float32r (TF32-ish split mode, 1 cycle/row at N>=256,
~1.5e-4 rel err) with fp32 PSUM accumulation; attention probs/V in bf16.
"""

import numpy as np

import concourse.bacc as bacc
import concourse.mybir as mybir
import concourse.tile as tile
from concourse.bass_utils import run_bass_kernel_spmd

F32 = mybir.dt.float32
F32R = mybir.dt.float32r
BF16 = mybir.dt.bfloat16
AF = mybir.ActivationFunctionType
OP = mybir.AluOpType

N_CORES = 8
S = 4          # sequences per core
T = 512        # sequence length
D = 512        # model dim (= e = n_head * d_k)
NH = 8         # heads
DV = 64        # head dim
C = 4          # 128-row chunks per 512 dim
P = 128
EPS = 1e-6

_PROGRAM_CACHE = {}


def _build_program(apply_affine: bool, loop_iters: int = 1):
    nc = bacc.Bacc()

    qT = nc.declare_dram_parameter("qT", [S, D, T], F32R, isOutput=False)
    kT = nc.declare_dram_parameter("kT", [S, D, T], F32R, isOutput=False)
    vT = nc.declare_dram_parameter("vT", [S, D, T], F32R, isOutput=False)
    qn = nc.declare_dram_parameter("qn", [S, T, D], F32, isOutput=False)
    wq = nc.declare_dram_parameter("wq", [D, D], F32R, isOutput=False)  # w_q.T
    wk = nc.declare_dram_parameter("wk", [D, D], F32R, isOutput=False)  # w_k.T
    wv = nc.declare_dram_parameter("wv", [D, D], F32R, isOutput=False)  # w_v.T
    wfc = nc.declare_dram_parameter("wfc", [D, D], F32R, isOutput=False)  # w_fc.T
    if apply_affine:
        gmb = nc.declare_dram_parameter("gmb", [P, D], F32, isOutput=False)
        btb = nc.declare_dram_parameter("btb", [P, D], F32, isOutput=False)
    out = nc.declare_dram_parameter("out", [S, T, D], F32, isOutput=True)

    with tile.TileContext(nc) as tc:
        with (
            tc.tile_pool(name="const", bufs=1) as cst,
            tc.tile_pool(name="inp", bufs=2) as inp,
            tc.tile_pool(name="proj", bufs=2) as proj,
            tc.tile_pool(name="expp", bufs=8) as expp,
            tc.tile_pool(name="xp", bufs=2) as xp,
            tc.tile_pool(name="small", bufs=2) as small,
            tc.tile_pool(name="psc", bufs=2, space="PSUM") as psc,
            tc.tile_pool(name="pfc", bufs=2, space="PSUM") as pfc,
            tc.tile_pool(name="pav", bufs=2, space="PSUM") as pavp,
        ):
            # one-time constants; weight DMAs split per 128-row chunk so the
            # first projection matmuls start as soon as chunk 0 lands.
            wq_sb = cst.tile([P, C, D], F32R, tag="wq")
            wk_sb = cst.tile([P, C, D], F32R, tag="wk")
            wv_sb = cst.tile([P, C, D], F32R, tag="wv")
            wfc_sb = cst.tile([P, C, D], F32R, tag="wfc")
            eps_sb = cst.tile([P, 1], F32, tag="eps")
            nc.vector.memset(eps_sb[:], EPS)
            if apply_affine:
                gm_sb = cst.tile([P, D], F32, tag="gmb")
                bt_sb = cst.tile([P, D], F32, tag="btb")
                nc.sync.dma_start(gm_sb[:], gmb[:])
                nc.sync.dma_start(bt_sb[:], btb[:])

            def load(s, weight_dmas=None):
                st = {}
                st["qT"] = inp.tile([P, C, T], F32R, tag="qT", name="qT_sb")
                st["kT"] = inp.tile([P, C, T], F32R, tag="kT", name="kT_sb")
                st["vT"] = inp.tile([P, C, T], F32R, tag="vT", name="vT_sb")
                # consumption order: (wq,qT) all chunks, then (wk,kT), (wv,vT)
                for (sb, dr), w_pair in zip(
                    ((st["qT"], qT), (st["kT"], kT), (st["vT"], vT)),
                    weight_dmas or ((), (), ()),
                ):
                    for dc in range(C):
                        for w_sb, w in w_pair:
                            nc.sync.dma_start(
                                w_sb[:, dc, :],
                                w.rearrange("(c p) e -> p c e", p=P)[:, dc, :],
                            )
                        nc.sync.dma_start(
                            sb[:, dc, :],
                            dr[s].rearrange("(c p) t -> p c t", p=P)[:, dc, :],
                        )
                return st

            def projA(s, st):
                # Q^T/K^T [e, t] head-major; V [t, e] with per-head ones col
                st["QT"] = proj.tile([P, C, T], F32R, tag="QT", name="QT_sb")
                st["KT"] = proj.tile([P, C, T], F32R, tag="KT", name="KT_sb")
                for dst, w_sb, x_sb in (
                    (st["QT"], wq_sb, st["qT"]), (st["KT"], wk_sb, st["kT"])
                ):
                    for ec in range(C):
                        ps = pfc.tile([P, T], F32, tag="fc", name="ps")
                        for dc in range(C):
                            nc.tensor.matmul(
                                ps[:],
                                lhsT=w_sb[:, dc, ec * P:(ec + 1) * P],
                                rhs=x_sb[:, dc, :],
                                start=(dc == 0),
                                stop=(dc == C - 1),
                            )
                        nc.vector.tensor_copy(dst[:, ec, :], ps[:])
                V_sb = proj.tile([P, C, NH, DV + 1], BF16, tag="V", name="V_sb")
                st["V"] = V_sb
                nc.gpsimd.memset(V_sb[:, :, :, DV:DV + 1], 1.0)
                for tc_ in range(C):
                    ps = pfc.tile([P, T], F32, tag="fc", name="ps")
                    for dc in range(C):
                        nc.tensor.matmul(
                            ps[:],
                            lhsT=st["vT"][:, dc, tc_ * P:(tc_ + 1) * P],
                            rhs=wv_sb[:, dc, :],
                            start=(dc == 0),
                            stop=(dc == C - 1),
                        )
                    nc.scalar.copy(
                        V_sb[:, tc_, :, 0:DV],
                        ps.rearrange("p (h v) -> p h v", h=NH),
                    )

            def attnB(s, st):
                # S^T = K_h Q_h^T with tk on partitions; heads are paired:
                # rows 0-63/64-127 of a KT/QT chunk are disjoint PE row
                # groups, so back-to-back K=64 matmuls run concurrently.
                # exp on ScalarE over [128,1024] 2-bank tiles (no max
                # subtraction; |S/8| <~ 7), bf16 out.
                expS = {}
                for hp in range(NH // 2):
                    for sub in range(2):
                        expS[2 * hp + sub] = expp.tile(
                            [P, C, T], BF16, tag="expS", name=f"expS_h"
                        )
                    for tg in range(2):
                        pss = []
                        for sub in range(2):
                            ps = psc.tile([P, 2 * T], F32, tag="sc", name="ps")
                            for dt_ in range(2):
                                tkc = 2 * tg + dt_
                                nc.tensor.matmul(
                                    ps[:, dt_ * T:(dt_ + 1) * T],
                                    lhsT=st["KT"][sub * DV:(sub + 1) * DV, hp,
                                                  tkc * P:(tkc + 1) * P],
                                    rhs=st["QT"][sub * DV:(sub + 1) * DV, hp, :],
                                    start=True,
                                    stop=True,
                                )
                            pss.append(ps)
                        for sub in range(2):
                            nc.scalar.activation(
                                expS[2 * hp + sub][:, 2 * tg:2 * tg + 2, :],
                                pss[sub][:], AF.Exp, scale=0.125,
                            )
                st["expS"] = expS

                # AV in natural orientation: for each tq chunk, 4 heads
                # accumulate into one PSUM bank (65 cols each incl. the
                # denominator from V's ones column); normalize+merge is one
                # strided multiply into x natural.
                x_nat = xp.tile([P, C, T], F32R, tag="xnat", name="x_nat")
                st["x"] = x_nat
                for half in range(2):
                    for tqc in range(C):
                        pav = pavp.tile([P, T], F32, tag="av", name="pav")
                        # each head's accumulation chain must be contiguous:
                        # start=True clears the has-written bits for the WHOLE
                        # bank, so interleaving chains would turn other heads'
                        # next accumulate into an overwrite.
                        for j in range(4):
                            h = 4 * half + j
                            for tkc in range(C):
                                nc.tensor.matmul(
                                    pav[:, j * (DV + 1):(j + 1) * (DV + 1)],
                                    lhsT=expS[h][:, tkc, tqc * P:(tqc + 1) * P],
                                    rhs=st["V"][:, tkc, h, :],
                                    start=(tkc == 0),
                                    stop=(tkc == C - 1),
                                )
                        pav_h = pav[:, 0:4 * (DV + 1)].rearrange(
                            "p (h v) -> p h v", h=4
                        )
                        R = small.tile([P, 4], F32, tag="R", bufs=4, name="R")
                        nc.vector.reciprocal(R[:], pav_h[:, :, DV])
                        nc.vector.tensor_tensor(
                            x_nat[:, tqc, half * 4 * DV:(half + 1) * 4 * DV]
                            .rearrange("p (h v) -> p h v", h=4),
                            pav_h[:, :, 0:DV],
                            R[:, :, None].to_broadcast((P, 4, DV)),
                            OP.mult,
                        )

            def tailC(s, st):
                # prefetch the residual rows early
                qn_cs = []
                for ac in range(C):
                    qn_c = small.tile([P, D], F32, tag="qn", bufs=4, name="qn_c")
                    nc.sync.dma_start(qn_c[:], qn[s, ac * P:(ac + 1) * P, :])
                    qn_cs.append(qn_c)
                st2_seq = small.tile([P, C, 2], F32, tag="st2", name="st2_seq")
                y_cs = []

                # fc (contracting over the *time* index, thanks to the
                # reference's transpose-view scramble) + residual + LayerNorm
                x_nat = st["x"]
                for ac in range(C):
                    psy = pfc.tile([P, T], F32, tag="fc", name="psy")
                    for cc in range(C):
                        nc.tensor.matmul(
                            psy[:],
                            lhsT=x_nat[:, cc, ac * P:(ac + 1) * P],
                            rhs=wfc_sb[:, cc, :],
                            start=(cc == 0),
                            stop=(cc == C - 1),
                        )
                    y_c = small.tile([P, D], F32, tag="y", bufs=4, name="y_c")
                    nc.vector.tensor_tensor(y_c[:], psy[:], qn_cs[ac][:], OP.add)
                    st6 = small.tile([P, 6], F32, tag="st6", name="st6")
                    nc.vector.bn_stats(st6[:], y_c[:])
                    nc.vector.bn_aggr(st2_seq[:, ac, :], st6[:])
                    y_cs.append(y_c)
                sd = small.tile([P, C], F32, tag="sd", name="sd")
                rinv = small.tile([P, C], F32, tag="rinv", name="rinv")
                nc.scalar.activation(sd[:], st2_seq[:, :, 1], AF.Sqrt, bias=eps_sb[:])
                nc.vector.reciprocal(rinv[:], sd[:])
                for ac in range(C):
                    y_c = y_cs[ac]
                    nc.vector.tensor_scalar(
                        y_c[:], y_c[:], st2_seq[:, ac, 0:1], rinv[:, ac:ac + 1],
                        OP.subtract, OP.mult,
                    )
                    if apply_affine:
                        nc.vector.tensor_tensor(y_c[:], y_c[:], gm_sb[:], OP.mult)
                        nc.vector.tensor_tensor(y_c[:], y_c[:], bt_sb[:], OP.add)
                    nc.sync.dma_start(out[s, ac * P:(ac + 1) * P, :], y_c[:])

            # software-pipelined emission: proj of seq s+1 is emitted before
            # the tail of seq s so the scheduler can fill PE gaps in the
            # attention/normalize phases with next-sequence matmuls.
            def emit_all():
                sts = {}
                sts[0] = load(0, weight_dmas=(
                    ((wq_sb, wq),),
                    ((wk_sb, wk),),
                    ((wv_sb, wv), (wfc_sb, wfc)),
                ))
                projA(0, sts[0])
                sts[1] = load(1)
                attnB(0, sts[0])
                for s in range(1, S):
                    projA(s, sts[s])
                    if s + 1 < S:
                        sts[s + 1] = load(s + 1)
                    tailC(s - 1, sts[s - 1])
                    attnB(s, sts[s])
                tailC(S - 1, sts[S - 1])

            if loop_iters == 1:
                emit_all()
            else:
                with tc.For_i(0, loop_iters, 1):
                    emit_all()

    nc.finalize()
    return nc


def _get_program(apply_affine: bool, loop_iters: int = 1):
    key = (apply_affine, loop_iters)
    if key not in _PROGRAM_CACHE:
        _PROGRAM_CACHE[key] = _build_program(apply_affine, loop_iters)
    return _PROGRAM_CACHE[key]


def kernel(q, k, v, w_q, w_k, w_v, w_fc, ln_gamma, ln_beta, _res_holder=None):
    q = np.asarray(q, dtype=np.float32)
    k = np.asarray(k, dtype=np.float32)
    v = np.asarray(v, dtype=np.float32)
    w_q = np.asarray(w_q, dtype=np.float32)
    w_k = np.asarray(w_k, dtype=np.float32)
    w_v = np.asarray(w_v, dtype=np.float32)
    w_fc = np.asarray(w_fc, dtype=np.float32)
    ln_gamma = np.asarray(ln_gamma, dtype=np.float32)
    ln_beta = np.asarray(ln_beta, dtype=np.float32)

    b, n, t, d = q.shape
    B = b * n
    assert (b, n, t, d) == (8, 4, T, D), q.shape
    qf = q.reshape(B, t, d)
    kf = k.reshape(B, t, d)
    vf = v.reshape(B, t, d)

    apply_affine = not (
        np.all(ln_gamma == 1.0) and np.all(ln_beta == 0.0)
    )
    nc = _get_program(apply_affine)

    wq_t = np.ascontiguousarray(w_q.T)
    wk_t = np.ascontiguousarray(w_k.T)
    wv_t = np.ascontiguousarray(w_v.T)
    wfc_t = np.ascontiguousarray(w_fc.T)

    in_maps = []
    for c in range(N_CORES):
        sl = slice(S * c, S * (c + 1))
        m = {
            "qT": np.ascontiguousarray(qf[sl].transpose(0, 2, 1)),
            "kT": np.ascontiguousarray(kf[sl].transpose(0, 2, 1)),
            "vT": np.ascontiguousarray(vf[sl].transpose(0, 2, 1)),
            "qn": np.ascontiguousarray(qf[sl]),
            "wq": wq_t, "wk": wk_t, "wv": wv_t, "wfc": wfc_t,
        }
        if apply_affine:
            m["gmb"] = np.ascontiguousarray(
                np.broadcast_to(ln_gamma, (P, D)).astype(np.float32)
            )
            m["btb"] = np.ascontiguousarray(
                np.broadcast_to(ln_beta, (P, D)).astype(np.float32)
            )
        in_maps.append(m)

    res = run_bass_kernel_spmd(nc, in_maps, list(range(N_CORES)))
    if _res_holder is not None:
        _res_holder.append(res)
    full = np.concatenate([res.results[c]["out"] for c in range(N_CORES)], axis=0)
    return full.reshape(b, n, t, d).astype(np.float32)


# revision 6
# speedup vs baseline: 9.8616x; 9.8616x over previous
"""Multi-head attention block (QKV proj + SDPA + merge-scramble + fc +
residual + LayerNorm) on 8 Trainium2 NeuronCores.

Sharding: data-parallel over the flattened batch dim (b*n = 32 sequences),
4 sequences per core. Each core runs an identical Bass program on its shard.

Per-sequence math (t = d = e = 512, H = 8 heads, dk = dv = 64):
  Q = qf @ w_q.T ; K = kf @ w_k.T ; V = vf @ w_v.T
  S_h = (Q_h K_h^T) / 8 ;  A_h = softmax(S_h) ;  O_h = A_h V_h
  x = merge_heads(O)            # [t, e]
  x = x.T (the reference's transpose+view scramble; legal since t == e)
  y = LN(x @ w_fc.T + qf) * gamma + beta

Design (validated on HW against simpler structures):
- Q^T/K^T [e, t] head-major on partitions; V [t, e] bf16 with a per-head
  ones column. S^T = K_h Q_h^T comes out with tk on partitions; heads are
  paired so the two K=64 score matmuls use disjoint PE row groups.
- exp runs on ScalarE over [128,1024] two-PSUM-bank tiles (no max
  subtraction needed; |S/8| < ~7), writing bf16.
- AV matmuls run in NATURAL orientation (lhsT = expS^T chunk [tk,tq],
  rhs = V_h||ones [tk,65]) producing O[tq,dv] packed 4 heads per PSUM
  bank; the ones column yields the softmax denominator per tq row in the
  same tile, so normalize+merge is one strided multiply into x natural —
  no PE transposes anywhere. The scramble means fc contracts over the
  *time* index, which is exactly x natural as lhsT.
- q/k/v and w_q/w_k/w_v stream in as bf16 (halves input DMA; rel err
  ~8e-4 vs the f32 reference). fc stays f32r on x and w_fc.
- Emission interleaves proj(s+1) and fc(s-1) matmul groups BETWEEN the
  score/exp stages of seq s: the scores<->exp PSUM ping-pong (2 score
  tiles in flight) is ACT-bound, and the interleave keeps PE busy with
  independent work instead of stalling on buffer reuse.
"""

import numpy as np
import ml_dtypes

import concourse.bacc as bacc
import concourse.mybir as mybir
import concourse.tile as tile
from concourse.bass_utils import run_bass_kernel_spmd

F32 = mybir.dt.float32
F32R = mybir.dt.float32r
BF16 = mybir.dt.bfloat16
AF = mybir.ActivationFunctionType
OP = mybir.AluOpType

N_CORES = 8
S = 4          # sequences per core
T = 512        # sequence length
D = 512        # model dim (= e = n_head * d_k)
NH = 8         # heads
DV = 64        # head dim
C = 4          # 128-row chunks per 512 dim
P = 128
EPS = 1e-6

_PROGRAM_CACHE = {}


def _build_program(apply_affine: bool, loop_iters: int = 1):
    nc = bacc.Bacc()

    qT = nc.declare_dram_parameter("qT", [S, D, T], BF16, isOutput=False)
    kT = nc.declare_dram_parameter("kT", [S, D, T], BF16, isOutput=False)
    vT = nc.declare_dram_parameter("vT", [S, D, T], BF16, isOutput=False)
    qn = nc.declare_dram_parameter("qn", [S, T, D], F32, isOutput=False)
    wq = nc.declare_dram_parameter("wq", [D, D], BF16, isOutput=False)  # w_q.T
    wk = nc.declare_dram_parameter("wk", [D, D], BF16, isOutput=False)  # w_k.T
    wv = nc.declare_dram_parameter("wv", [D, D], BF16, isOutput=False)  # w_v.T
    wfc = nc.declare_dram_parameter("wfc", [D, D], F32R, isOutput=False)  # w_fc.T
    if apply_affine:
        gmb = nc.declare_dram_parameter("gmb", [P, D], F32, isOutput=False)
        btb = nc.declare_dram_parameter("btb", [P, D], F32, isOutput=False)
    out = nc.declare_dram_parameter("out", [S, T, D], F32, isOutput=True)

    with tile.TileContext(nc) as tc:
        with (
            tc.tile_pool(name="const", bufs=1) as cst,
            tc.tile_pool(name="inp", bufs=2) as inp,
            tc.tile_pool(name="proj", bufs=2) as proj,
            tc.tile_pool(name="expp", bufs=8) as expp,
            tc.tile_pool(name="xp", bufs=2) as xp,
            tc.tile_pool(name="small", bufs=2) as small,
            tc.tile_pool(name="psc", bufs=2, space="PSUM") as psc,
            tc.tile_pool(name="pfc", bufs=2, space="PSUM") as pfc,
            tc.tile_pool(name="pav", bufs=2, space="PSUM") as pavp,
        ):
            wq_sb = cst.tile([P, C, D], BF16, tag="wq")
            wk_sb = cst.tile([P, C, D], BF16, tag="wk")
            wv_sb = cst.tile([P, C, D], BF16, tag="wv")
            wfc_sb = cst.tile([P, C, D], F32R, tag="wfc")
            eps_sb = cst.tile([P, 1], F32, tag="eps")
            nc.vector.memset(eps_sb[:], EPS)
            if apply_affine:
                gm_sb = cst.tile([P, D], F32, tag="gmb")
                bt_sb = cst.tile([P, D], F32, tag="btb")
                nc.sync.dma_start(gm_sb[:], gmb[:])
                nc.sync.dma_start(bt_sb[:], btb[:])

            def load(s, weight_dmas=None):
                st = {}
                st["qT"] = inp.tile([P, C, T], BF16, tag="qT", name="qT_sb")
                st["kT"] = inp.tile([P, C, T], BF16, tag="kT", name="kT_sb")
                st["vT"] = inp.tile([P, C, T], BF16, tag="vT", name="vT_sb")
                # consumption order: (wq,qT) all chunks, then (wk,kT), (wv,vT)
                for (sb, dr), w_pair in zip(
                    ((st["qT"], qT), (st["kT"], kT), (st["vT"], vT)),
                    weight_dmas or ((), (), ()),
                ):
                    for dc in range(C):
                        for w_sb, w in w_pair:
                            nc.sync.dma_start(
                                w_sb[:, dc, :],
                                w.rearrange("(c p) e -> p c e", p=P)[:, dc, :],
                            )
                        nc.sync.dma_start(
                            sb[:, dc, :],
                            dr[s].rearrange("(c p) t -> p c t", p=P)[:, dc, :],
                        )
                return st

            def proj_groups(s, st):
                """12 thunks: QT ec0-3, KT ec0-3, V tc0-3; drains on DVE."""
                st["QT"] = proj.tile([P, C, T], F32R, tag="QT", name="QT_sb")
                st["KT"] = proj.tile([P, C, T], F32R, tag="KT", name="KT_sb")
                V_sb = proj.tile([P, C, NH, DV + 1], BF16, tag="V", name="V_sb")
                st["V"] = V_sb
                groups = []

                def mk_qk(dst, w_sb, x_sb, ec):
                    def run():
                        ps = pfc.tile([P, T], F32, tag="fc", name="ps")
                        for dc in range(C):
                            nc.tensor.matmul(
                                ps[:],
                                lhsT=w_sb[:, dc, ec * P:(ec + 1) * P],
                                rhs=x_sb[:, dc, :],
                                start=(dc == 0),
                                stop=(dc == C - 1),
                            )
                        nc.vector.tensor_copy(dst[:, ec, :], ps[:])
                    return run

                def mk_v(tc_):
                    def run():
                        if tc_ == 0:
                            nc.gpsimd.memset(V_sb[:, :, :, DV:DV + 1], 1.0)
                        ps = pfc.tile([P, T], F32, tag="fc", name="ps")
                        for dc in range(C):
                            nc.tensor.matmul(
                                ps[:],
                                lhsT=st["vT"][:, dc, tc_ * P:(tc_ + 1) * P],
                                rhs=wv_sb[:, dc, :],
                                start=(dc == 0),
                                stop=(dc == C - 1),
                            )
                        nc.vector.tensor_copy(
                            V_sb[:, tc_, :, 0:DV],
                            ps.rearrange("p (h v) -> p h v", h=NH),
                        )
                    return run

                for ec in range(C):
                    groups.append(mk_qk(st["QT"], wq_sb, st["qT"], ec))
                for ec in range(C):
                    groups.append(mk_qk(st["KT"], wk_sb, st["kT"], ec))
                for tc_ in range(C):
                    groups.append(mk_v(tc_))
                return groups

            def scores_stages(s, st):
                """8 thunks; stage (hp, tg): 2 wide score MMs (the two heads
                of pair hp use disjoint PE row groups) + 2 [128,1024] exps."""
                st["expS"] = expS = {}
                stages = []

                def mk(hp, tg):
                    def run():
                        if tg == 0:
                            for sub in range(2):
                                expS[2 * hp + sub] = expp.tile(
                                    [P, C, T], BF16, tag="expS", name="expS_h"
                                )
                        pss = []
                        for sub in range(2):
                            ps = psc.tile([P, 2 * T], F32, tag="sc", name="ps")
                            for dt_ in range(2):
                                tkc = 2 * tg + dt_
                                nc.tensor.matmul(
                                    ps[:, dt_ * T:(dt_ + 1) * T],
                                    lhsT=st["KT"][sub * DV:(sub + 1) * DV, hp,
                                                  tkc * P:(tkc + 1) * P],
                                    rhs=st["QT"][sub * DV:(sub + 1) * DV, hp, :],
                                    start=True,
                                    stop=True,
                                )
                            pss.append(ps)
                        for sub in range(2):
                            nc.scalar.activation(
                                expS[2 * hp + sub][:, 2 * tg:2 * tg + 2, :],
                                pss[sub][:], AF.Exp, scale=0.125,
                            )
                    return run

                for hp in range(NH // 2):
                    for tg in range(2):
                        stages.append(mk(hp, tg))
                return stages

            def av_groups(s, st):
                """8 thunks: (half, tqc) — 4 heads' accumulation chains into
                one PSUM bank + one fused normalize/merge into x natural.
                Each head's chain is contiguous: start=True clears the
                has-written bits for the WHOLE bank, so interleaving chains
                would corrupt other heads' accumulation."""
                expS = st["expS"]
                x_nat = xp.tile([P, C, T], F32R, tag="xnat", name="x_nat")
                st["x"] = x_nat
                groups = []

                def mk(half, tqc):
                    def run():
                        pav = pavp.tile([P, T], F32, tag="av", name="pav")
                        for j in range(4):
                            h = 4 * half + j
                            for tkc in range(C):
                                nc.tensor.matmul(
                                    pav[:, j * (DV + 1):(j + 1) * (DV + 1)],
                                    lhsT=expS[h][:, tkc, tqc * P:(tqc + 1) * P],
                                    rhs=st["V"][:, tkc, h, :],
                                    start=(tkc == 0),
                                    stop=(tkc == C - 1),
                                )
                        pav_h = pav[:, 0:4 * (DV + 1)].rearrange(
                            "p (h v) -> p h v", h=4
                        )
                        R = small.tile([P, 4], F32, tag="R", bufs=4, name="R")
                        nc.vector.reciprocal(R[:], pav_h[:, :, DV])
                        nc.vector.tensor_tensor(
                            x_nat[:, tqc, half * 4 * DV:(half + 1) * 4 * DV]
                            .rearrange("p (h v) -> p h v", h=4),
                            pav_h[:, :, 0:DV],
                            R[:, :, None].to_broadcast((P, 4, DV)),
                            OP.mult,
                        )
                    return run

                for half in range(2):
                    for tqc in range(C):
                        groups.append(mk(half, tqc))
                return groups

            def fc_groups(s, st):
                """5 thunks: 4x (fc chunk + residual + bn), then LN + store.
                fc contracts over the *time* index (the reference's
                transpose-view scramble), i.e. lhsT = x natural."""
                qn_cs = []
                st2_seq = small.tile([P, C, 2], F32, tag="st2", name="st2_seq")
                y_cs = []
                groups = []

                def mk_ac(ac):
                    def run():
                        qn_c = small.tile([P, D], F32, tag="qn", bufs=4,
                                          name="qn_c")
                        nc.sync.dma_start(qn_c[:], qn[s, ac * P:(ac + 1) * P, :])
                        qn_cs.append(qn_c)
                        psy = pfc.tile([P, T], F32, tag="fc", name="psy")
                        for cc in range(C):
                            nc.tensor.matmul(
                                psy[:],
                                lhsT=st["x"][:, cc, ac * P:(ac + 1) * P],
                                rhs=wfc_sb[:, cc, :],
                                start=(cc == 0),
                                stop=(cc == C - 1),
                            )
                        y_c = small.tile([P, D], F32, tag="y", bufs=8,
                                         name="y_c")
                        nc.vector.tensor_tensor(y_c[:], psy[:], qn_c[:], OP.add)
                        st6 = small.tile([P, 6], F32, tag="st6", name="st6")
                        nc.vector.bn_stats(st6[:], y_c[:])
                        nc.vector.bn_aggr(st2_seq[:, ac, :], st6[:])
                        y_cs.append(y_c)
                    return run

                def ln_out():
                    sd = small.tile([P, C], F32, tag="sd", name="sd")
                    rinv = small.tile([P, C], F32, tag="rinv", name="rinv")
                    nc.scalar.activation(sd[:], st2_seq[:, :, 1], AF.Sqrt,
                                         bias=eps_sb[:])
                    nc.vector.reciprocal(rinv[:], sd[:])
                    for ac in range(C):
                        y_c = y_cs[ac]
                        nc.vector.tensor_scalar(
                            y_c[:], y_c[:], st2_seq[:, ac, 0:1],
                            rinv[:, ac:ac + 1], OP.subtract, OP.mult,
                        )
                        if apply_affine:
                            nc.vector.tensor_tensor(y_c[:], y_c[:], gm_sb[:],
                                                    OP.mult)
                            nc.vector.tensor_tensor(y_c[:], y_c[:], bt_sb[:],
                                                    OP.add)
                        nc.sync.dma_start(out[s, ac * P:(ac + 1) * P, :], y_c[:])

                for ac in range(C):
                    groups.append(mk_ac(ac))
                groups.append(ln_out)
                return groups

            def emit_body():
                sts = {0: load(0, weight_dmas=(
                    ((wq_sb, wq),),
                    ((wk_sb, wk),),
                    ((wv_sb, wv), (wfc_sb, wfc)),
                ))}
                for g in proj_groups(0, sts[0]):
                    g()
                for s in range(S):
                    if s + 1 < S:
                        sts[s + 1] = load(s + 1)
                    A = scores_stages(s, sts[s])
                    B = (list(proj_groups(s + 1, sts[s + 1]))
                         if s + 1 < S else [])
                    Cg = list(fc_groups(s - 1, sts[s - 1])) if s >= 1 else []
                    for i in range(len(A)):
                        A[i]()
                        if B:
                            B.pop(0)()
                        if i % 2 == 1 and B:
                            B.pop(0)()
                        if i % 2 == 0 and Cg:
                            Cg.pop(0)()
                    D = av_groups(s, sts[s])
                    for i, d in enumerate(D):
                        d()
                        if B:
                            B.pop(0)()
                        if Cg:
                            Cg.pop(0)()
                    while B:
                        B.pop(0)()
                    while Cg:
                        Cg.pop(0)()
                for g in fc_groups(S - 1, sts[S - 1]):
                    g()

            if loop_iters == 1:
                emit_body()
            else:
                with tc.For_i(0, loop_iters, 1):
                    emit_body()

    nc.finalize()
    return nc


def _get_program(apply_affine: bool, loop_iters: int = 1):
    key = (apply_affine, loop_iters)
    if key not in _PROGRAM_CACHE:
        _PROGRAM_CACHE[key] = _build_program(apply_affine, loop_iters)
    return _PROGRAM_CACHE[key]


def kernel(q, k, v, w_q, w_k, w_v, w_fc, ln_gamma, ln_beta, _res_holder=None):
    q = np.asarray(q, dtype=np.float32)
    k = np.asarray(k, dtype=np.float32)
    v = np.asarray(v, dtype=np.float32)
    w_q = np.asarray(w_q, dtype=np.float32)
    w_k = np.asarray(w_k, dtype=np.float32)
    w_v = np.asarray(w_v, dtype=np.float32)
    w_fc = np.asarray(w_fc, dtype=np.float32)
    ln_gamma = np.asarray(ln_gamma, dtype=np.float32)
    ln_beta = np.asarray(ln_beta, dtype=np.float32)

    b, n, t, d = q.shape
    B = b * n
    assert (b, n, t, d) == (8, 4, T, D), q.shape
    qf = q.reshape(B, t, d)
    kf = k.reshape(B, t, d)
    vf = v.reshape(B, t, d)

    apply_affine = not (
        np.all(ln_gamma == 1.0) and np.all(ln_beta == 0.0)
    )
    nc = _get_program(apply_affine)

    bf = ml_dtypes.bfloat16
    wq_t = np.ascontiguousarray(w_q.T).astype(bf)
    wk_t = np.ascontiguousarray(w_k.T).astype(bf)
    wv_t = np.ascontiguousarray(w_v.T).astype(bf)
    wfc_t = np.ascontiguousarray(w_fc.T)

    in_maps = []
    for c in range(N_CORES):
        sl = slice(S * c, S * (c + 1))
        m = {
            "qT": np.ascontiguousarray(qf[sl].transpose(0, 2, 1)).astype(bf),
            "kT": np.ascontiguousarray(kf[sl].transpose(0, 2, 1)).astype(bf),
            "vT": np.ascontiguousarray(vf[sl].transpose(0, 2, 1)).astype(bf),
            "qn": np.ascontiguousarray(qf[sl]),
            "wq": wq_t, "wk": wk_t, "wv": wv_t, "wfc": wfc_t,
        }
        if apply_affine:
            m["gmb"] = np.ascontiguousarray(
                np.broadcast_to(ln_gamma, (P, D)).astype(np.float32)
            )
            m["btb"] = np.ascontiguousarray(
                np.broadcast_to(ln_beta, (P, D)).astype(np.float32)
            )
        in_maps.append(m)

    res = run_bass_kernel_spmd(nc, in_maps, list(range(N_CORES)))
    if _res_holder is not None:
        _res_holder.append(res)
    full = np.concatenate([res.results[c]["out"] for c in range(N_CORES)], axis=0)
    return full.reshape(b, n, t, d).astype(np.float32)
